# revision 25
# baseline (speedup 1.0000x reference)
"""DGCNN (3x DynamicEdgeConv + global max pool + FC) Trainium2 Bass kernel.

Sharding: data-parallel over graphs. 32 graphs / 8 NeuronCores = 4 graphs/core.
Weights replicated. Each core returns its [128, 4] (feature-major) FC output.

Per-graph algorithm (feature-major [C, P] layout end to end):
  - kNN ranking matrix F = X^T X - 0.5*|x_j|^2 via one PE matmul with the
    lhs=[X;ones], rhs=[X;-0.5 sq] augmentation (top-20 largest F == 20-NN).
  - Top-20 indices per node: 3 rounds of DVE max8 / max_index / match_replace.
  - EdgeConv decomposes: relu(max_k([x_i, x_j-x_i] W + b))
      = relu((Wtop-Wbot)^T x_i + max_k Wbot^T x_j + b)  (relu/max commute).
    So per node: A = Wd^T X (PE), Bm = X^T Wbot rows in DRAM, M = max over the
    20 neighbor rows via 20 indirect DMA gathers with max-accumulate.
  - h^T = relu(transpose(M) + A + b) using PE transpose + matmul accumulated
    into one PSUM tile, ACT applies relu+bias.

HW note: indirect DMA consumes the offset AP partition-fastest, so each gather
uses one k-column of idx ([128, 8] AP = all 1024 nodes in node order) and the
[128, 8, H] gather output holds node 8p+t at [p, t]; downstream blocks use
node-strided (t::8) access patterns to compensate.
"""
import sys

sys.path.insert(0, "/opt/trn_rl_repo")
import numpy as np
import concourse.bass as bass
import concourse.bacc as bacc
import concourse.mybir as mybir
from concourse.bass_utils import run_bass_kernel_spmd
from concourse.tile import TileContext
from concourse import masks

B, P, KNN = 32, 1024, 20
NCORES, GPC = 8, 4
NEG = -3.0e38
f32, u32 = mybir.dt.float32, mybir.dt.uint32
RELU = mybir.ActivationFunctionType.Relu
COPY = mybir.ActivationFunctionType.Copy
MAX = mybir.AluOpType.max
DIMS = {1: (3, 64), 2: (64, 64), 3: (64, 128)}

_cache = {}


def _emit_layer(nc, tc, pools, W, state, g, l, is_last):
    C, H = DIMS[l]
    lhs, rhs = state[(g, "lhs")], state[(g, "rhs")]
    wd, wb, bl = W[f"wd{l}"], W[f"wb{l}"], W[f"b{l}"]
    ident, diagneg, onescol = W["ident"], W["diagneg"], W["onescol"]
    psF, psT, psB = pools["psF"], pools["psT"], pools["psB"]
    pool = pools["sbuf"]
    bm_dram = state[(g, "bm64")] if H == 64 else state[(g, "bm128")]

    # ---- 1. ones row + sq row (layer 1 rows are shipped from host) ----
    if l > 1:
        _emit_sq_prep(nc, pools, W, lhs, rhs, C)
    _emit_layer_rest(nc, tc, pools, W, state, g, l, is_last)


def _emit_sq_prep(nc, pools, W, lhs, rhs, C):
    psF = pools["psF"]
    pool = pools["sbuf"]
    onescol = W["onescol"]
    nc.vector.memset(lhs[C:C + 1, :], 1.0)
    x2 = pool.tile([C, P], f32, tag="x2", bufs=1)
    nc.scalar.square(x2[0:C, :], lhs[0:C, :])
    for jb in range(2):
        psq = psF.tile([128, 512], f32, tag="psF")
        nc.tensor.matmul(psq[0:1, :], onescol[0:C, :],
                         x2[0:C, 512 * jb:512 * (jb + 1)], start=True, stop=True)
        nc.scalar.activation(rhs[C:C + 1, 512 * jb:512 * (jb + 1)], psq[0:1, :],
                             COPY, scale=-0.5)


def _emit_layer_rest(nc, tc, pools, W, state, g, l, is_last):
    C, H = DIMS[l]
    lhs, rhs = state[(g, "lhs")], state[(g, "rhs")]
    wd, wb, bl = W[f"wd{l}"], W[f"wb{l}"], W[f"b{l}"]
    ident, diagneg, onescol = W["ident"], W["diagneg"], W["onescol"]
    psF, psT, psB = pools["psF"], pools["psT"], pools["psB"]
    pool = pools["sbuf"]
    bm_dram = state[(g, "bm64")] if H == 64 else state[(g, "bm128")]

    # ---- 2. Bm = X^T Wbot, node-major to DRAM ----
    bmt = pool.tile([128, 8, 128], f32, tag="bm", bufs=2)
    for t in range(8):
        pb = psB.tile([128, 128], f32, tag="psB")
        nc.tensor.matmul(pb[:, 0:H], lhs[0:C, 128 * t:128 * (t + 1)], wb[0:C, 0:H],
                         start=True, stop=True)
        nc.scalar.activation(bmt[:, t, 0:H], pb[:, 0:H], COPY)
    nc.sync.dma_start(out=bm_dram[:].rearrange("(t p) h -> p t h", p=128), in_=bmt[:, :, 0:H])

    # ---- 3. F + top-20 indices per node-tile ----
    idxs = pool.tile([128, 8, 24], u32, tag="idx", bufs=3)
    for t in range(8):
        Fsb = pool.tile([128, P], f32, tag="F", bufs=6)
        for jb in range(2):
            fps = psF.tile([128, 512], f32, tag="psF")
            nc.tensor.matmul(fps[:], lhs[0:C + 1, 128 * t:128 * (t + 1)],
                             rhs[0:C + 1, 512 * jb:512 * (jb + 1)],
                             start=True, stop=True)
            nc.scalar.activation(Fsb[:, 512 * jb:512 * (jb + 1)], fps[:], COPY)
        nc.vector.tensor_add(Fsb[:, 128 * t:128 * (t + 1)],
                             Fsb[:, 128 * t:128 * (t + 1)], diagneg[:])
        for r in range(3):
            m8 = pool.tile([128, 8], f32, tag="m8", bufs=4)
            nc.vector.max(out=m8, in_=Fsb)
            nc.vector.max_index(out=idxs[:, t, 8 * r:8 * r + 8], in_max=m8,
                                in_values=Fsb)
            if r < 2:
                nc.vector.match_replace(out=Fsb, in_to_replace=m8, in_values=Fsb,
                                        imm_value=NEG)

    # ---- 4+5. per-row-tile: 20 single-descriptor-per-partition gathers ----
    if is_last:
        h3 = pool.tile([128, P], f32, tag="h3", bufs=1)
        dst = h3
    else:
        Cn = H + 1
        lhs_n = pool.tile([Cn, P], f32, tag=f"lhs{l + 1}", bufs=4)
        rhs_n = pool.tile([Cn, P], f32, tag=f"rhs{l + 1}", bufs=4)
        dst = lhs_n
    for t in range(8):
        wt = pool.tile([128, 24], u32, tag="wt", bufs=4, name=f"wt{t}")
        nc.vector.tensor_copy(wt[:], idxs[:, t, :])
        gt = pool.tile([128, KNN, H], f32, tag="gt", bufs=3, name=f"gt{t}")
        for k in range(KNN):
            nc.gpsimd.indirect_dma_start(
                out=gt[:, k, :], out_offset=None,
                in_=bm_dram[:, :],
                in_offset=bass.IndirectOffsetOnAxis(ap=wt[:, k:k + 1], axis=0),
                bounds_check=P - 1, oob_is_err=False)
        Mt = pool.tile([128, H], f32, tag="Mt", bufs=4, name=f"Mt{t}")
        nc.vector.tensor_reduce(
            out=Mt[:], in_=gt[:].rearrange("p c h -> p h c"),
            axis=mybir.AxisListType.X, op=MAX)
        pt = psT.tile([128, 128], f32, tag="psT")
        nc.tensor.matmul(pt[0:H, :], Mt[:], ident[:], is_transpose=True,
                         start=True, stop=False)
        nc.tensor.matmul(pt[0:H, :], wd[0:C, 0:H],
                         lhs[0:C, 128 * t:128 * (t + 1)], start=False, stop=True)
        nc.scalar.activation(dst[0:H, 128 * t:128 * (t + 1)], pt[0:H, :], RELU,
                             bias=bl[0:H, :])

    if is_last:
        nc.vector.tensor_reduce(out=W["pooledT"][:, g:g + 1], in_=h3[:],
                                axis=mybir.AxisListType.X, op=MAX)
    else:
        nc.gpsimd.tensor_copy(rhs_n[0:H, :], lhs_n[0:H, :])
        state[(g, "lhs")], state[(g, "rhs")] = lhs_n, rhs_n


def _build():
    nc = bacc.Bacc("TRN2", target_bir_lowering=False, debug=False,
                   num_devices=NCORES)
    xa_in = nc.declare_dram_parameter("xa", [GPC, 4, P], f32, isOutput=False)
    xb_in = nc.declare_dram_parameter("xb", [GPC, 4, P], f32, isOutput=False)
    params = {}
    for l, (C, H) in DIMS.items():
        params[f"wd{l}"] = nc.declare_dram_parameter(f"wd{l}", [C, H], f32, isOutput=False)
        params[f"wb{l}"] = nc.declare_dram_parameter(f"wb{l}", [C, H], f32, isOutput=False)
        params[f"b{l}"] = nc.declare_dram_parameter(f"b{l}", [H, 1], f32, isOutput=False)
    wfc_in = nc.declare_dram_parameter("wfc", [128, 128], f32, isOutput=False)
    bfc_in = nc.declare_dram_parameter("bfc", [128, 1], f32, isOutput=False)
    ptab_in = nc.declare_dram_parameter("ptab", [8, 128, 24], u32, isOutput=False)
    out_d = nc.declare_dram_parameter("out", [128, GPC], f32, isOutput=True)


    state = {}
    for g in range(GPC):
        state[(g, "bm64")] = nc.dram_tensor(f"bm64_{g}", [P, 64], f32)
        state[(g, "bm128")] = nc.dram_tensor(f"bm128_{g}", [P, 128], f32)

    with TileContext(nc) as tc:
        with tc.tile_pool(name="consts", bufs=1) as consts, \
             tc.tile_pool(name="weights", bufs=1) as wpool, \
             tc.tile_pool(name="sbuf", bufs=2) as sbuf, \
             tc.tile_pool(name="psF", bufs=3, space="PSUM") as psF, \
             tc.tile_pool(name="psT", bufs=3, space="PSUM") as psT, \
             tc.tile_pool(name="psB", bufs=2, space="PSUM") as psB:
            pools = {"sbuf": sbuf, "psF": psF, "psT": psT, "psB": psB}
            W = {}
            W["ident"] = consts.tile([128, 128], f32, name="ident")
            masks.make_identity(nc, W["ident"][:])
            W["diagneg"] = consts.tile([128, 128], f32, name="diagneg")
            nc.gpsimd.memset(W["diagneg"][:], 0.0)
            nc.gpsimd.affine_select(
                out=W["diagneg"][:], in_=W["diagneg"][:],
                compare_op=mybir.AluOpType.not_equal, fill=NEG,
                base=0, pattern=[[-1, 128]], channel_multiplier=1)
            W["onescol"] = consts.tile([128, 1], f32, name="onescol")
            nc.vector.memset(W["onescol"][:], 1.0)
            W["pooledT"] = consts.tile([128, GPC], f32, name="pooledT")
            for l, (C, H) in DIMS.items():
                for nm, shp in ((f"wd{l}", [C, H]), (f"wb{l}", [C, H]),
                                (f"b{l}", [H, 1])):
                    tl = wpool.tile(shp, f32, tag=nm, name=nm)
                    nc.sync.dma_start(out=tl[:], in_=params[nm][:, :])
                    W[nm] = tl
            wfc = wpool.tile([128, 128], f32, tag="wfc")
            nc.sync.dma_start(out=wfc[:], in_=wfc_in[:, :])
            bfc = wpool.tile([128, 1], f32, tag="bfc")
            nc.sync.dma_start(out=bfc[:], in_=bfc_in[:, :])

            for g in range(GPC):
                lhs1 = sbuf.tile([4, P], f32, tag="lhs1", bufs=4)
                rhs1 = sbuf.tile([4, P], f32, tag="rhs1", bufs=4)
                nc.sync.dma_start(out=lhs1[:, :], in_=xa_in[g, :, :])
                nc.sync.dma_start(out=rhs1[:, :], in_=xb_in[g, :, :])
                state[(g, "lhs")], state[(g, "rhs")] = lhs1, rhs1

            for l in (1, 2, 3):
                for g in range(GPC):
                    _emit_layer(nc, tc, pools, W, state, g, l, is_last=(l == 3))

            ptf = psT.tile([128, 128], f32, tag="psT")
            nc.tensor.matmul(ptf[:, 0:GPC], wfc[:], W["pooledT"][:, 0:GPC],
                             start=True, stop=True)
            outsb = sbuf.tile([128, GPC], f32, tag="outsb")
            nc.scalar.activation(outsb[:], ptf[:, 0:GPC], RELU, bias=bfc[:])
            nc.sync.dma_start(out=out_d[:, :], in_=outsb[:])

    nc.compile()
    return nc


def _get_nc():
    if "nc" not in _cache:
        _cache["nc"] = _build()
    return _cache["nc"]


def _prep_in_maps(inputs):
    x = np.ascontiguousarray(np.asarray(inputs["x"], dtype=np.float32))
    x = x.reshape(B, P, 3)
    shared = {}
    for l, (C, H) in DIMS.items():
        Wl = np.asarray(inputs[f"W{l}"], dtype=np.float32)
        bl = np.asarray(inputs[f"b{l}"], dtype=np.float32)
        shared[f"wd{l}"] = np.ascontiguousarray(Wl[:C] - Wl[C:])
        shared[f"wb{l}"] = np.ascontiguousarray(Wl[C:])
        shared[f"b{l}"] = np.ascontiguousarray(bl[:, None])
    shared["wfc"] = np.ascontiguousarray(np.asarray(inputs["Wfc"], dtype=np.float32))
    shared["bfc"] = np.ascontiguousarray(
        np.asarray(inputs["bfc"], dtype=np.float32)[:, None])
    xt = x.transpose(0, 2, 1)  # [B, 3, P]
    ones = np.ones((B, 1, P), np.float32)
    sqr = -0.5 * (xt * xt).sum(axis=1, keepdims=True)
    xa = np.concatenate([xt, ones], axis=1)   # [B, 4, P]
    xb = np.concatenate([xt, sqr], axis=1)    # [B, 4, P]
    ptab = np.zeros((8, 128, 24), np.uint32)
    for q in range(128):
        for s in range(KNN):
            j2 = 20 * q + s
            node = (128 * s + q) // 20
            k = (128 * s + q) % 20
            for t in range(8):
                ptab[t, j2 % 128, j2 // 128] = node * 24 + k + 3072 * t
    shared["ptab"] = ptab
    in_maps = []
    for c in range(NCORES):
        m = dict(shared)
        m["xa"] = np.ascontiguousarray(xa[GPC * c:GPC * (c + 1)])
        m["xb"] = np.ascontiguousarray(xb[GPC * c:GPC * (c + 1)])
        in_maps.append(m)
    return in_maps


def _run(inputs, trace=False):
    nc = _get_nc()
    in_maps = _prep_in_maps(inputs)
    res = run_bass_kernel_spmd(nc, in_maps, list(range(NCORES)), trace=trace)
    out = np.concatenate([res.results[c]["out"].T for c in range(NCORES)], axis=0)
    return out.astype(np.float32), res


def kernel(**inputs):
    out, _ = _run(inputs, trace=False)
    return out


# revision 26
# speedup vs baseline: 1.2266x; 1.2266x over previous
"""DGCNN (3x DynamicEdgeConv + global max pool + FC) Trainium2 Bass kernel.

Sharding: data-parallel over graphs. 32 graphs / 8 NeuronCores = 4 graphs/core.
Weights replicated. Each core returns its [128, 4] (feature-major) FC output.

Per-graph algorithm (feature-major [C, P] layout end to end):
  - kNN ranking matrix F = X^T X - 0.5*|x_j|^2 via one PE matmul with the
    lhs=[X;ones], rhs=[X;-0.5 sq] augmentation (top-20 largest F == 20-NN).
  - Top-20 indices per node: 3 rounds of DVE max8 / max_index / match_replace.
  - EdgeConv decomposes: relu(max_k([x_i, x_j-x_i] W + b))
      = relu((Wtop-Wbot)^T x_i + max_k Wbot^T x_j + b)  (relu/max commute).
    So per node: A = Wd^T X (PE), Bm = X^T Wbot rows in DRAM, M = max over the
    20 neighbor rows via 20 indirect DMA gathers with max-accumulate.
  - h^T = relu(transpose(M) + A + b) using PE transpose + matmul accumulated
    into one PSUM tile, ACT applies relu+bias.

HW note: multi-column indirect-DMA offset APs are consumed in a scrambled
order on this hardware, so each gather uses a [128, 1] offset column (one
descriptor per partition — unambiguous, production-tested shape): 20 gathers
per 128-node row-tile into k-slices of a [128, 20, H] tile, then one DVE
tensor_reduce(max) over k.
"""
import sys

sys.path.insert(0, "/opt/trn_rl_repo")
import numpy as np
import concourse.bass as bass
import concourse.bacc as bacc
import concourse.mybir as mybir
from concourse.bass_utils import run_bass_kernel_spmd
from concourse.tile import TileContext
from concourse import masks

B, P, KNN = 32, 1024, 20
NCORES, GPC = 8, 4
NEG = -3.0e38
f32, u32 = mybir.dt.float32, mybir.dt.uint32
RELU = mybir.ActivationFunctionType.Relu
COPY = mybir.ActivationFunctionType.Copy
MAX = mybir.AluOpType.max
DIMS = {1: (3, 64), 2: (64, 64), 3: (64, 128)}

_cache = {}


def _emit_layer(nc, tc, pools, W, state, g, l, is_last):
    C, H = DIMS[l]
    lhs, rhs = state[(g, "lhs")], state[(g, "rhs")]
    wd, wb, bl = W[f"wd{l}"], W[f"wb{l}"], W[f"b{l}"]
    ident, diagneg, onescol = W["ident"], W["diagneg"], W["onescol"]
    psF, psT, psB = pools["psF"], pools["psT"], pools["psB"]
    pool = pools["sbuf"]
    bm_dram = state[(g, "bm64")] if H == 64 else state[(g, "bm128")]

    # ---- 1. ones row + sq row (layer 1 rows are shipped from host) ----
    if l > 1:
        _emit_sq_prep(nc, pools, W, lhs, rhs, C)
    _emit_layer_rest(nc, tc, pools, W, state, g, l, is_last)


def _emit_sq_prep(nc, pools, W, lhs, rhs, C):
    psF = pools["psF"]
    pool = pools["sbuf"]
    onescol = W["onescol"]
    nc.vector.memset(lhs[C:C + 1, :], 1.0)
    x2 = pool.tile([C, P], f32, tag="x2", bufs=1)
    nc.scalar.square(x2[0:C, :], lhs[0:C, :])
    for jb in range(2):
        psq = psF.tile([128, 512], f32, tag="psF")
        nc.tensor.matmul(psq[0:1, :], onescol[0:C, :],
                         x2[0:C, 512 * jb:512 * (jb + 1)], start=True, stop=True)
        nc.scalar.activation(rhs[C:C + 1, 512 * jb:512 * (jb + 1)], psq[0:1, :],
                             COPY, scale=-0.5)


def _emit_layer_rest(nc, tc, pools, W, state, g, l, is_last):
    C, H = DIMS[l]
    lhs, rhs = state[(g, "lhs")], state[(g, "rhs")]
    wd, wb, bl = W[f"wd{l}"], W[f"wb{l}"], W[f"b{l}"]
    ident, diagneg, onescol = W["ident"], W["diagneg"], W["onescol"]
    psF, psT, psB = pools["psF"], pools["psT"], pools["psB"]
    pool = pools["sbuf"]
    bm_dram = state[(g, "bm64")] if H == 64 else state[(g, "bm128")]

    # ---- 2. Bm = X^T Wbot, node-major to DRAM ----
    bmt = pool.tile([128, 8, 128], f32, tag="bm", bufs=2)
    for t in range(8):
        pb = psB.tile([128, 128], f32, tag="psB")
        nc.tensor.matmul(pb[:, 0:H], lhs[0:C, 128 * t:128 * (t + 1)], wb[0:C, 0:H],
                         start=True, stop=True)
        nc.scalar.activation(bmt[:, t, 0:H], pb[:, 0:H], COPY)
    nc.sync.dma_start(out=bm_dram[:].rearrange("(t p) h -> p t h", p=128), in_=bmt[:, :, 0:H])

    # ---- 3. F + top-20 indices per node-tile ----
    idxs = pool.tile([128, 8, 24], u32, tag="idx", bufs=3)
    for t in range(8):
        Fsb = pool.tile([128, P], f32, tag="F", bufs=6)
        for jb in range(2):
            fps = psF.tile([128, 512], f32, tag="psF")
            nc.tensor.matmul(fps[:], lhs[0:C + 1, 128 * t:128 * (t + 1)],
                             rhs[0:C + 1, 512 * jb:512 * (jb + 1)],
                             start=True, stop=True)
            nc.scalar.activation(Fsb[:, 512 * jb:512 * (jb + 1)], fps[:], COPY)
        nc.vector.tensor_add(Fsb[:, 128 * t:128 * (t + 1)],
                             Fsb[:, 128 * t:128 * (t + 1)], diagneg[:])
        for r in range(3):
            m8 = pool.tile([128, 8], f32, tag="m8", bufs=4)
            nc.vector.max(out=m8, in_=Fsb)
            nc.vector.max_index(out=idxs[:, t, 8 * r:8 * r + 8], in_max=m8,
                                in_values=Fsb)
            if r < 2:
                nc.vector.match_replace(out=Fsb, in_to_replace=m8, in_values=Fsb,
                                        imm_value=NEG)

    # ---- 4+5. per-row-tile: 20 single-descriptor-per-partition gathers ----
    if is_last:
        h3 = pool.tile([128, P], f32, tag="h3", bufs=1)
        dst = h3
    else:
        Cn = H + 1
        lhs_n = pool.tile([Cn, P], f32, tag=f"lhs{l + 1}", bufs=4)
        rhs_n = pool.tile([Cn, P], f32, tag=f"rhs{l + 1}", bufs=4)
        dst = lhs_n
    for t in range(8):
        wt = pool.tile([128, 24], u32, tag="wt", bufs=4, name=f"wt{t}")
        nc.vector.tensor_copy(wt[:], idxs[:, t, :])
        gt = pool.tile([128, KNN, H], f32, tag="gt", bufs=3, name=f"gt{t}")
        for k in range(KNN):
            nc.gpsimd.indirect_dma_start(
                out=gt[:, k, :], out_offset=None,
                in_=bm_dram[:, :],
                in_offset=bass.IndirectOffsetOnAxis(ap=wt[:, k:k + 1], axis=0),
                bounds_check=P - 1, oob_is_err=False)
        Mt = pool.tile([128, H], f32, tag="Mt", bufs=4, name=f"Mt{t}")
        nc.vector.tensor_reduce(
            out=Mt[:], in_=gt[:].rearrange("p c h -> p h c"),
            axis=mybir.AxisListType.X, op=MAX)
        pt = psT.tile([128, 128], f32, tag="psT")
        nc.tensor.matmul(pt[0:H, :], Mt[:], ident[:], is_transpose=True,
                         start=True, stop=False)
        nc.tensor.matmul(pt[0:H, :], wd[0:C, 0:H],
                         lhs[0:C, 128 * t:128 * (t + 1)], start=False, stop=True)
        nc.scalar.activation(dst[0:H, 128 * t:128 * (t + 1)], pt[0:H, :], RELU,
                             bias=bl[0:H, :])

    if is_last:
        nc.vector.tensor_reduce(out=W["pooledT"][:, g:g + 1], in_=h3[:],
                                axis=mybir.AxisListType.X, op=MAX)
    else:
        nc.gpsimd.tensor_copy(rhs_n[0:H, :], lhs_n[0:H, :])
        state[(g, "lhs")], state[(g, "rhs")] = lhs_n, rhs_n


def _build():
    nc = bacc.Bacc("TRN2", target_bir_lowering=False, debug=False,
                   num_devices=NCORES)
    xa_in = nc.declare_dram_parameter("xa", [GPC, 4, P], f32, isOutput=False)
    xb_in = nc.declare_dram_parameter("xb", [GPC, 4, P], f32, isOutput=False)
    params = {}
    for l, (C, H) in DIMS.items():
        params[f"wd{l}"] = nc.declare_dram_parameter(f"wd{l}", [C, H], f32, isOutput=False)
        params[f"wb{l}"] = nc.declare_dram_parameter(f"wb{l}", [C, H], f32, isOutput=False)
        params[f"b{l}"] = nc.declare_dram_parameter(f"b{l}", [H, 1], f32, isOutput=False)
    wfc_in = nc.declare_dram_parameter("wfc", [128, 128], f32, isOutput=False)
    bfc_in = nc.declare_dram_parameter("bfc", [128, 1], f32, isOutput=False)
    ptab_in = nc.declare_dram_parameter("ptab", [8, 128, 24], u32, isOutput=False)
    out_d = nc.declare_dram_parameter("out", [128, GPC], f32, isOutput=True)


    state = {}
    for g in range(GPC):
        state[(g, "bm64")] = nc.dram_tensor(f"bm64_{g}", [P, 64], f32)
        state[(g, "bm128")] = nc.dram_tensor(f"bm128_{g}", [P, 128], f32)

    with TileContext(nc) as tc:
        with tc.tile_pool(name="consts", bufs=1) as consts, \
             tc.tile_pool(name="weights", bufs=1) as wpool, \
             tc.tile_pool(name="sbuf", bufs=2) as sbuf, \
             tc.tile_pool(name="psF", bufs=3, space="PSUM") as psF, \
             tc.tile_pool(name="psT", bufs=3, space="PSUM") as psT, \
             tc.tile_pool(name="psB", bufs=2, space="PSUM") as psB:
            pools = {"sbuf": sbuf, "psF": psF, "psT": psT, "psB": psB}
            W = {}
            W["ident"] = consts.tile([128, 128], f32, name="ident")
            masks.make_identity(nc, W["ident"][:])
            W["diagneg"] = consts.tile([128, 128], f32, name="diagneg")
            nc.gpsimd.memset(W["diagneg"][:], 0.0)
            nc.gpsimd.affine_select(
                out=W["diagneg"][:], in_=W["diagneg"][:],
                compare_op=mybir.AluOpType.not_equal, fill=NEG,
                base=0, pattern=[[-1, 128]], channel_multiplier=1)
            W["onescol"] = consts.tile([128, 1], f32, name="onescol")
            nc.vector.memset(W["onescol"][:], 1.0)
            W["pooledT"] = consts.tile([128, GPC], f32, name="pooledT")
            for l, (C, H) in DIMS.items():
                for nm, shp in ((f"wd{l}", [C, H]), (f"wb{l}", [C, H]),
                                (f"b{l}", [H, 1])):
                    tl = wpool.tile(shp, f32, tag=nm, name=nm)
                    nc.sync.dma_start(out=tl[:], in_=params[nm][:, :])
                    W[nm] = tl
            wfc = wpool.tile([128, 128], f32, tag="wfc")
            nc.sync.dma_start(out=wfc[:], in_=wfc_in[:, :])
            bfc = wpool.tile([128, 1], f32, tag="bfc")
            nc.sync.dma_start(out=bfc[:], in_=bfc_in[:, :])

            for g in range(GPC):
                lhs1 = sbuf.tile([4, P], f32, tag="lhs1", bufs=4)
                rhs1 = sbuf.tile([4, P], f32, tag="rhs1", bufs=4)
                nc.sync.dma_start(out=lhs1[:, :], in_=xa_in[g, :, :])
                nc.sync.dma_start(out=rhs1[:, :], in_=xb_in[g, :, :])
                state[(g, "lhs")], state[(g, "rhs")] = lhs1, rhs1

            for l in (1, 2, 3):
                for g in range(GPC):
                    _emit_layer(nc, tc, pools, W, state, g, l, is_last=(l == 3))

            ptf = psT.tile([128, 128], f32, tag="psT")
            nc.tensor.matmul(ptf[:, 0:GPC], wfc[:], W["pooledT"][:, 0:GPC],
                             start=True, stop=True)
            outsb = sbuf.tile([128, GPC], f32, tag="outsb")
            nc.scalar.activation(outsb[:], ptf[:, 0:GPC], RELU, bias=bfc[:])
            nc.sync.dma_start(out=out_d[:, :], in_=outsb[:])

    nc.compile()
    return nc


def _get_nc():
    if "nc" not in _cache:
        _cache["nc"] = _build()
    return _cache["nc"]


def _prep_in_maps(inputs):
    x = np.ascontiguousarray(np.asarray(inputs["x"], dtype=np.float32))
    x = x.reshape(B, P, 3)
    shared = {}
    for l, (C, H) in DIMS.items():
        Wl = np.asarray(inputs[f"W{l}"], dtype=np.float32)
        bl = np.asarray(inputs[f"b{l}"], dtype=np.float32)
        shared[f"wd{l}"] = np.ascontiguousarray(Wl[:C] - Wl[C:])
        shared[f"wb{l}"] = np.ascontiguousarray(Wl[C:])
        shared[f"b{l}"] = np.ascontiguousarray(bl[:, None])
    shared["wfc"] = np.ascontiguousarray(np.asarray(inputs["Wfc"], dtype=np.float32))
    shared["bfc"] = np.ascontiguousarray(
        np.asarray(inputs["bfc"], dtype=np.float32)[:, None])
    xt = x.transpose(0, 2, 1)  # [B, 3, P]
    ones = np.ones((B, 1, P), np.float32)
    sqr = -0.5 * (xt * xt).sum(axis=1, keepdims=True)
    xa = np.concatenate([xt, ones], axis=1)   # [B, 4, P]
    xb = np.concatenate([xt, sqr], axis=1)    # [B, 4, P]
    ptab = np.zeros((8, 128, 24), np.uint32)
    for q in range(128):
        for s in range(KNN):
            j2 = 20 * q + s
            node = (128 * s + q) // 20
            k = (128 * s + q) % 20
            for t in range(8):
                ptab[t, j2 % 128, j2 // 128] = node * 24 + k + 3072 * t
    shared["ptab"] = ptab
    in_maps = []
    for c in range(NCORES):
        m = dict(shared)
        m["xa"] = np.ascontiguousarray(xa[GPC * c:GPC * (c + 1)])
        m["xb"] = np.ascontiguousarray(xb[GPC * c:GPC * (c + 1)])
        in_maps.append(m)
    return in_maps


def _run(inputs, trace=False):
    nc = _get_nc()
    in_maps = _prep_in_maps(inputs)
    res = run_bass_kernel_spmd(nc, in_maps, list(range(NCORES)), trace=trace)
    out = np.concatenate([res.results[c]["out"].T for c in range(NCORES)], axis=0)
    return out.astype(np.float32), res


def kernel(**inputs):
    out, _ = _run(inputs, trace=False)
    return out
